# revision 21
# baseline (speedup 1.0000x reference)
"""Multi-head attention (B=2, T=2048, D=2048, H=16) on 8 trn2 NeuronCores.

Sharding: tensor-parallel over heads. Core c owns heads {2c, 2c+1}:
  - QKV projection for its 2 heads (Q^T/K^T in [dh, t] layout, V in [t, dh]).
  - Attention per (head, batch), computed as S^T = K^T.T Q^T so softmax probs
    land in [s, t] layout and feed the AV matmul directly (no transposes).
    Softmax skips max-subtraction (scores are O(15) here; exp stays well
    inside fp32 range). The denominator is tree-accumulated on DVE in bf16,
    partition-reduced by a ones-matvec on PE, reciprocated at [1, t] and
    broadcast back over partitions with a rank-1 PE matmul.
  - Two per-head AllToAlls redistribute attention outputs from head-sharded
    [dh, t] blocks to row-sharded x^T [D, 512] per core; the first overlaps
    with the second head's attention, the second hides under the hh=0 half
    of the output projection (which only needs AllToAll #1's data). The xr
    staging is split into 8 contiguous per-core DMAs on the gpsimd queue so
    no strided descriptor storm sits on the critical path.
  - Each core then computes 512 rows of the output projection with the full
    W_out, preloaded into SBUF during phase B when DMA is otherwise idle.
Host assembles the 8 row-shards (each returned transposed) into [B, T, D].

All matmul operands are bf16: same PE rate as f32r but 1.5x faster
LDWEIGHTS, half the DMA/SBUF footprint, and 2x DVE throughput for the
softmax-denominator tree; accumulation stays fp32 in PSUM. Startup DMAs are
JIT-ordered across the sync/scalar queues in first-chain consumption order
(the first ~30us are DMA-bandwidth-bound).
"""

import numpy as np

D = 2048
H = 16
DH = 128
B = 2
T = 2048
NT = B * T            # 4096 flattened rows
NCORES = 8
HPC = H // NCORES     # heads per core = 2
ROWS = NT // NCORES   # output rows per core = 512
NFT = D // 128        # 16 feature tiles
SCALE = float(np.sqrt(np.float32(DH)) / np.sqrt(np.float32(D)))  # 0.25

_CACHE = {}


def _build():
    from contextlib import ExitStack

    import concourse.bass as bass  # noqa: F401
    import concourse.mybir as mybir
    import concourse.tile as tile
    from concourse import bacc

    f32 = mybir.dt.float32
    f32r = mybir.dt.float32r
    bf16 = mybir.dt.bfloat16
    Act = mybir.ActivationFunctionType

    nc = bacc.Bacc("TRN2", target_bir_lowering=False, debug=False,
                   num_devices=NCORES)

    ACH = 512                # phase-A t-chunk width
    NACH = NT // ACH         # 8 chunks

    # inputs are pre-arranged on the host into SBUF-image layouts so every
    # load is a fully contiguous per-partition DMA
    xT = nc.dram_tensor("xT", [128, NACH, NFT, ACH], bf16,
                        kind="ExternalInput")
    wqkv = nc.dram_tensor("wqkv", [128, 6, NFT, DH], bf16,
                          kind="ExternalInput")
    bqkv = nc.dram_tensor("bqkv", [6, DH], f32, kind="ExternalInput")
    wout = nc.dram_tensor("wout", [128, 8, NFT, 256], bf16,
                          kind="ExternalInput")
    bout = nc.dram_tensor("bout", [D], f32, kind="ExternalInput")
    outT = nc.dram_tensor("outT", [D, ROWS], f32, kind="ExternalOutput")
    BCH = 512                # phase-B t-chunk width
    NBCH = T // BCH          # 4 chunks per (head, batch)
    NST = T // 128           # 16 s-tiles per batch

    with tile.TileContext(nc) as tc, ExitStack() as es:
        persist = es.enter_context(tc.tile_pool(name="persist", bufs=1))
        dram = es.enter_context(tc.tile_pool(name="dram", bufs=1,
                                             space="DRAM"))
        a2a_in = [dram.tile([NCORES, DH, ROWS], bf16, name=f"a2a_in{h}")
                  for h in range(HPC)]
        a2a_out = [dram.tile([NCORES, DH, ROWS], bf16, name=f"a2a_out{h}")
                   for h in range(HPC)]

        bqkv_sb = persist.tile([128, 6], f32)
        bv_sb = persist.tile([128, HPC * DH], f32)
        bout_sb = persist.tile([128, NFT], f32)
        ones_sb = persist.tile([128, 128], f32)
        ones_b = persist.tile([128, 128], bf16)
        ones_r = persist.tile([128, 128], f32r)

        nc.vector.memset(ones_sb, 1.0)
        nc.vector.tensor_copy(ones_b, ones_sb)
        nc.vector.tensor_copy(ones_r, ones_sb)

        xr0_sb = [persist.tile([128, ROWS], bf16, name=f"xr0_{c}")
                  for c in range(NCORES)]

        pcB = es.enter_context(tc.tile_pool(name="pcB", bufs=1))
        xr1_sb = [pcB.tile([128, ROWS], bf16, name=f"xr1_{c}")
                  for c in range(NCORES)]
        partial_sb = pcB.tile([128, NFT, ROWS], bf16)
        xr_sb = [xr0_sb, xr1_sb]

        mid = es.enter_context(tc.tile_pool(name="mid", bufs=1))
        qt_sb = [mid.tile([128, NT], bf16, name=f"qt{h}") for h in range(HPC)]
        kt_sb = [mid.tile([128, NT], bf16, name=f"kt{h}") for h in range(HPC)]
        v_sb = [mid.tile([128, B, NST, DH], bf16, name=f"v{h}")
                for h in range(HPC)]

        # ---------------- Phase A: QKV projection ----------------
        with tc.tile_pool(name="phaseA", bufs=1) as pa, \
             tc.tile_pool(name="xtp", bufs=2) as xtp, \
             tc.tile_pool(name="psA", bufs=2, space="PSUM") as psA:
            wqkv_sb = pa.tile([128, 6, NFT, DH], bf16)
            # JIT-ordered startup: interleave the first x chunk's pieces with
            # the weight slots in consumption order across both DMA queues
            nc.sync.dma_start(out=wqkv_sb[:, 0, 0:8, :],
                              in_=wqkv[:, 0, 0:8, :])

            first_q_act = None
            for tch in range(NACH):
                t0 = tch * ACH
                xt_tile = xtp.tile([128, NFT, ACH], bf16, tag="xt")
                if tch == 0:
                    for q in range(8):
                        eng = nc.scalar if q % 2 == 0 else nc.sync
                        eng.dma_start(
                            out=xt_tile[:, 2 * q:2 * q + 2, :],
                            in_=xT[:, 0, 2 * q:2 * q + 2, :])
                        if q == 1:
                            nc.sync.dma_start(out=wqkv_sb[:, 0, 8:16, :],
                                              in_=wqkv[:, 0, 8:16, :])
                        elif q == 3:
                            nc.sync.dma_start(out=wqkv_sb[:, 2, :, :],
                                              in_=wqkv[:, 2, :, :])
                            nc.sync.dma_start(
                                out=bqkv_sb, in_=bqkv[:, :].transpose([1, 0]))
                        elif q == 5:
                            nc.scalar.dma_start(out=wqkv_sb[:, 1, :, :],
                                                in_=wqkv[:, 1, :, :])
                        elif q == 7:
                            nc.scalar.dma_start(out=wqkv_sb[:, 3, :, :],
                                                in_=wqkv[:, 3, :, :])
                    nc.sync.dma_start(out=wqkv_sb[:, 4, :, :],
                                      in_=wqkv[:, 4, :, :])
                    nc.scalar.dma_start(out=wqkv_sb[:, 5, :, :],
                                        in_=wqkv[:, 5, :, :])
                    nc.sync.dma_start(out=bv_sb,
                                      in_=bqkv[4:6, :].flatten().unsqueeze(0)
                                      .to_broadcast([128, HPC * DH]))
                    nc.sync.dma_start(out=bout_sb,
                                      in_=bout.rearrange("(n p) -> p n",
                                                         p=128))
                else:
                    dma = nc.gpsimd.dma_start(out=xt_tile,
                                              in_=xT[:, tch, :, :])
                    if tch == 1 and first_q_act is not None:
                        # keep the chunk-1 prefetch off the DMA engines while
                        # chunk 0's critical pieces stream in
                        from concourse.bass import _add_dep_helper
                        _add_dep_helper(dma.ins, first_q_act.ins, sync=False,
                                        reason="xt1 after first Q chain")
                for h in range(HPC):
                    ps_q = psA.tile([128, ACH], f32, tag="psq")
                    for ft in range(NFT):
                        nc.tensor.matmul(
                            ps_q,
                            wqkv_sb[:, h, ft, :], xt_tile[:, ft, :],
                            start=(ft == 0), stop=(ft == NFT - 1))
                    act = nc.scalar.activation(
                        out=qt_sb[h][:, t0:t0 + ACH], in_=ps_q,
                        func=Act.Identity, bias=bqkv_sb[:, h:h + 1])
                    if tch == 0 and h == 0 and first_q_act is None:
                        first_q_act = act
                    ps_k = psA.tile([128, ACH], f32, tag="psk")
                    for ft in range(NFT):
                        nc.tensor.matmul(
                            ps_k,
                            wqkv_sb[:, 2 + h, ft, :], xt_tile[:, ft, :],
                            start=(ft == 0), stop=(ft == NFT - 1))
                    nc.scalar.activation(
                        out=kt_sb[h][:, t0:t0 + ACH], in_=ps_k,
                        func=Act.Identity, bias=bqkv_sb[:, 2 + h:3 + h])
                for st in range(ACH // 128):
                    ps_v = psA.tile([128, HPC * DH], f32, tag="psv")
                    for ft in range(NFT):
                        nc.tensor.matmul(
                            ps_v,
                            xt_tile[:, ft, st * 128:(st + 1) * 128],
                            wqkv_sb[:, 4:6, ft, :],
                            start=(ft == 0), stop=(ft == NFT - 1))
                    g = t0 + st * 128
                    b_idx, st_b = g // T, (g % T) // 128
                    for h in range(HPC):
                        nc.vector.tensor_add(
                            v_sb[h][:, b_idx, st_b, :],
                            ps_v[:, h * DH:(h + 1) * DH],
                            bv_sb[:, h * DH:(h + 1) * DH])

        wp_pool = es.enter_context(tc.tile_pool(name="wp", bufs=1))
        wpiece = [wp_pool.tile([128, NFT, 256], bf16, name=f"wp{p}")
                  for p in range(8)]
        for p in range(8):
            nc.gpsimd.dma_start(out=wpiece[p], in_=wout[:, p, :, :])

        def emit_c_chain(psc_pool, hh, fti, outp=None):
            ps_c = psc_pool.tile([128, ROWS], f32, tag="psc",
                                 name=f"psc{hh}_{fti}")
            for c in range(NCORES):
                nc.tensor.matmul(
                    ps_c,
                    wpiece[fti // 2][:, HPC * c + hh,
                                     (fti % 2) * 128:
                                     (fti % 2) * 128 + 128],
                    xr_sb[hh][c],
                    start=(c == 0), stop=(c == NCORES - 1))
            if hh == 0:
                nc.scalar.activation(out=partial_sb[:, fti, :],
                                     in_=ps_c, func=Act.Identity,
                                     bias=bout_sb[:, fti:fti + 1])
            else:
                out_sb = outp.tile([128, ROWS], f32, tag="ob")
                nc.vector.tensor_add(out_sb, ps_c,
                                     partial_sb[:, fti, :])
                nc.sync.dma_start(
                    out=outT[fti * 128:(fti + 1) * 128, :],
                    in_=out_sb)

        # ---------------- Phase B: attention (h outer, split A2A) --------
        # Software pipeline: chunk k's AV matmuls interleave with chunk k+1's
        # S matmuls so PE stays busy while ACT works through the exps.
        with tc.tile_pool(name="ptp", bufs=2) as ptp, \
             tc.tile_pool(name="wkB", bufs=3) as wkB, \
             tc.tile_pool(name="psS", bufs=2, space="PSUM") as psS, \
             tc.tile_pool(name="psO", bufs=3, space="PSUM") as psO, \
             tc.tile_pool(name="psX", bufs=1, space="PSUM") as psX:

            def emit_av(pend, st_list):
                h, b, j, pt_p = pend["h"], pend["b"], pend["j"], pend["pt"]
                if pend["ps_o"] is None:
                    ps_o = psO.tile([128, BCH], f32, tag="pso",
                                    name=f"pso{h}_{j}")
                    pend["ps_o"] = ps_o
                for st in st_list:
                    nc.tensor.matmul(
                        pend["ps_o"],
                        v_sb[h][:, b, st, :], pt_p[:, st, :],
                        start=(st == 0), stop=(st == NST - 1))

            def emit_tree_step(pend, step):
                h, j, pt_p = pend["h"], pend["j"], pend["pt"]
                if step == 0:
                    acc4 = wkB.tile([128, 4, BCH], bf16, tag="acc4", bufs=2,
                                    name=f"acc4_{h}_{j}")
                    pend["acc4"] = acc4
                acc4 = pend["acc4"]
                if step == 0:
                    nc.vector.tensor_add(acc4, pt_p[:, 0:4, :],
                                         pt_p[:, 4:8, :])
                elif step == 1:
                    nc.vector.tensor_add(acc4, acc4, pt_p[:, 8:12, :])
                elif step == 2:
                    nc.vector.tensor_add(acc4, acc4, pt_p[:, 12:16, :])
                elif step == 3:
                    nc.vector.tensor_add(acc4[:, 0:2, :], acc4[:, 0:2, :],
                                         acc4[:, 2:4, :])
                else:
                    nc.vector.tensor_add(acc4[:, 0, :], acc4[:, 0, :],
                                         acc4[:, 1, :])

            def emit_epilogue(pend):
                h, b, j, pt_p = pend["h"], pend["b"], pend["j"], pend["pt"]
                for step in range(pend["tree_step"], 5):
                    emit_tree_step(pend, step)
                acc4 = pend["acc4"]
                ps_den = psX.tile([128, BCH], f32, tag="psx",
                                  name=f"psden{h}_{j}")
                nc.tensor.matmul(ps_den[0:1, :], ones_b[:, 0:1],
                                 acc4[:, 0, :],
                                 start=True, stop=True)
                den_sb = wkB.tile([1, BCH], f32, tag="densb",
                                  name=f"den{h}_{j}")
                nc.vector.reciprocal_approx_fast(den_sb[0:1, :],
                                                 ps_den[0:1, :])
                # broadcast 1/den across partitions via a DRAM bounce on the
                # idle DMA path instead of a rank-1 PE matmul + PSUM copy
                den_dram = dram.tile([BCH], f32, name=f"dend{h}_{j}")
                nc.sync.dma_start(out=den_dram, in_=den_sb[0:1, :])
                rb_sb = wkB.tile([128, BCH], f32, tag="rb",
                                 name=f"rb{h}_{j}")
                nc.sync.dma_start(
                    out=rb_sb,
                    in_=den_dram.unsqueeze(0).to_broadcast([128, BCH]))
                obuf = wkB.tile([128, BCH], bf16, tag="obuf",
                                name=f"obuf{h}_{j}")
                nc.vector.tensor_mul(obuf, pend["ps_o"], rb_sb)
                nc.sync.dma_start(out=a2a_in[h][j, :, :], in_=obuf)

            def emit_a2a(h):
                nc.gpsimd.collective_compute(
                    "AllToAll", mybir.AluOpType.bypass,
                    replica_groups=[list(range(NCORES))],
                    ins=[a2a_in[h].opt()], outs=[a2a_out[h].opt()])
                if h == 0:
                    # xr0 staging runs under head 1's attention; the gpsimd
                    # queue is idle in phase B so its wait on A2A#1 blocks
                    # nothing. Per-core pieces keep each DMA contiguous.
                    for c in range(NCORES):
                        nc.gpsimd.dma_start(out=xr0_sb[c],
                                            in_=a2a_out[0][c, :, :])

            pending = None
            for h in range(HPC):
                for b in range(B):
                    for tc2 in range(NBCH):
                        t0 = b * T + tc2 * BCH
                        pt = ptp.tile([128, NST, BCH], bf16, tag="pt",
                                      name=f"pt{h}_{b}_{tc2}")
                        cur = {"h": h, "b": b, "j": b * NBCH + tc2,
                               "pt": pt, "ps_o": None, "tree_step": 0}
                        for sg in range(NST // 2):
                            ps_s = psS.tile([128, 2, BCH], f32, tag="pss",
                                            name=f"pss{h}_{b}_{tc2}_{sg}")
                            for si in range(2):
                                st = sg * 2 + si
                                nc.tensor.matmul(
                                    ps_s[:, si, :],
                                    kt_sb[h][:,
                                             b * T + st * 128:
                                             b * T + (st + 1) * 128],
                                    qt_sb[h][:, t0:t0 + BCH],
                                    start=True, stop=True)
                            nc.scalar.activation(
                                out=pt[:, sg * 2:(sg + 1) * 2, :],
                                in_=ps_s, func=Act.Exp, scale=SCALE)
                            if pending is not None:
                                emit_av(pending, [sg * 2, sg * 2 + 1])
                                if sg >= 3 and pending["tree_step"] < 5:
                                    emit_tree_step(pending,
                                                   pending["tree_step"])
                                    pending["tree_step"] += 1
                            last = (h == HPC - 1 and b == B - 1
                                    and tc2 == NBCH - 1)
                            if last and sg > 0:
                                # drain chunk: AV rides one batch behind exp
                                emit_av(cur, [sg * 2 - 2, sg * 2 - 1])
                        if pending is not None:
                            emit_epilogue(pending)
                            if pending["h"] == 0 and \
                                    pending["j"] == B * NBCH - 1:
                                # head 0 fully drained: its AllToAll issues
                                # here, one chunk into head 1's attention, so
                                # h1's S matmuls are never queued behind the
                                # epilogue matvecs
                                emit_a2a(0)
                        pending = cur
            # final drain and head-1 collective
            emit_av(pending, [NST - 2, NST - 1])
            emit_epilogue(pending)
            emit_a2a(1)
        # ---------------- Phase C: output projection ----------------
        outp = es.enter_context(tc.tile_pool(name="outp", bufs=3))
        psC = es.enter_context(tc.tile_pool(name="psC", bufs=8, space="PSUM"))
        for c in range(NCORES):
            nc.gpsimd.dma_start(out=xr1_sb[c], in_=a2a_out[1][c, :, :])
        for hh in range(HPC):
            for fti in range(NFT):
                emit_c_chain(psC, hh, fti, outp)

    nc.compile()
    return nc


def get_nc():
    if "nc" not in _CACHE:
        _CACHE["nc"] = _build()
    return _CACHE["nc"]


def make_in_maps(query, W_qkv, b_qkv, W_out, b_out):
    import ml_dtypes
    bf = ml_dtypes.bfloat16
    query = np.asarray(query, dtype=np.float32)
    W_qkv = np.asarray(W_qkv, dtype=np.float32)
    b_qkv = np.asarray(b_qkv, dtype=np.float32)
    W_out_bf = (np.asarray(W_out, dtype=np.float32)
                .astype(bf)
                .reshape(NFT, 128, 8, 256).transpose(1, 2, 0, 3))
    W_out_bf = np.ascontiguousarray(W_out_bf)  # [128, 8, NFT, 256]
    b_out = np.ascontiguousarray(np.asarray(b_out, dtype=np.float32))

    ACH = 512
    x = query.reshape(NT, D).astype(bf)
    xT = x.T.reshape(NFT, 128, NT // ACH, ACH).transpose(1, 2, 0, 3)
    xT = np.ascontiguousarray(xT)          # [128, NACH, NFT, ACH]

    in_maps = []
    for c in range(NCORES):
        h0, h1 = HPC * c, HPC * c + 1
        cols, bcols = [], []
        for blk in range(3):  # q, k, v blocks of W_qkv
            for h in (h0, h1):
                sl = slice(blk * D + h * DH, blk * D + (h + 1) * DH)
                cols.append(W_qkv[:, sl])
                bcols.append(b_qkv[sl])
        wq = (np.stack(cols, axis=0)            # [6, D, DH]
              .reshape(6, NFT, 128, DH).transpose(2, 0, 1, 3))
        in_maps.append({
            "xT": xT,
            "wqkv": np.ascontiguousarray(wq.astype(bf)),  # [128, 6, NFT, DH]
            "bqkv": np.ascontiguousarray(np.stack(bcols, axis=0)),
            "wout": W_out_bf,
            "bout": b_out,
        })
    return in_maps


def kernel(query, key, value, W_qkv, b_qkv, W_out, b_out):
    from concourse.bass_utils import run_bass_kernel_spmd

    nc = get_nc()
    in_maps = make_in_maps(query, W_qkv, b_qkv, W_out, b_out)
    res = run_bass_kernel_spmd(nc, in_maps, list(range(NCORES)))
    out = np.empty((NT, D), dtype=np.float32)
    for c in range(NCORES):
        out[c * ROWS:(c + 1) * ROWS, :] = res.results[c]["outT"].T
    return out.reshape(B, T, D)


# revision 22
# speedup vs baseline: 1.0345x; 1.0345x over previous
"""Multi-head attention (B=2, T=2048, D=2048, H=16) on 8 trn2 NeuronCores.

Sharding: tensor-parallel over heads. Core c owns heads {2c, 2c+1}:
  - QKV projection for its 2 heads (Q^T/K^T in [dh, t] layout, V in [t, dh]).
  - Attention per (head, batch), computed as S^T = K^T.T Q^T so softmax probs
    land in [s, t] layout and feed the AV matmul directly (no transposes).
    Softmax skips max-subtraction (scores are O(15) here; exp stays well
    inside fp32 range). The denominator is tree-accumulated on DVE in bf16,
    partition-reduced by a ones-matvec on PE, reciprocated at [1, t] and
    broadcast back over partitions with a rank-1 PE matmul.
  - Two per-head AllToAlls redistribute attention outputs from head-sharded
    [dh, t] blocks to row-sharded x^T [D, 512] per core; the first overlaps
    with the second head's attention, the second hides under the hh=0 half
    of the output projection (which only needs AllToAll #1's data). The xr
    staging is split into 8 contiguous per-core DMAs on the gpsimd queue so
    no strided descriptor storm sits on the critical path.
  - Each core then computes 512 rows of the output projection with the full
    W_out, preloaded into SBUF during phase B when DMA is otherwise idle.
Host assembles the 8 row-shards (each returned transposed) into [B, T, D].

All matmul operands are bf16: same PE rate as f32r but 1.5x faster
LDWEIGHTS, half the DMA/SBUF footprint, and 2x DVE throughput for the
softmax-denominator tree; accumulation stays fp32 in PSUM. Startup DMAs are
JIT-ordered across the sync/scalar queues in first-chain consumption order
(the first ~30us are DMA-bandwidth-bound).
"""

import numpy as np

D = 2048
H = 16
DH = 128
B = 2
T = 2048
NT = B * T            # 4096 flattened rows
NCORES = 8
HPC = H // NCORES     # heads per core = 2
ROWS = NT // NCORES   # output rows per core = 512
NFT = D // 128        # 16 feature tiles
SCALE = float(np.sqrt(np.float32(DH)) / np.sqrt(np.float32(D)))  # 0.25

_CACHE = {}


def _build():
    from contextlib import ExitStack

    import concourse.bass as bass  # noqa: F401
    import concourse.mybir as mybir
    import concourse.tile as tile
    from concourse import bacc

    f32 = mybir.dt.float32
    f32r = mybir.dt.float32r
    bf16 = mybir.dt.bfloat16
    Act = mybir.ActivationFunctionType

    nc = bacc.Bacc("TRN2", target_bir_lowering=False, debug=False,
                   num_devices=NCORES)

    ACH = 512                # phase-A t-chunk width
    NACH = NT // ACH         # 8 chunks

    # inputs are pre-arranged on the host into SBUF-image layouts so every
    # load is a fully contiguous per-partition DMA
    xT = nc.dram_tensor("xT", [128, NACH, NFT, ACH], bf16,
                        kind="ExternalInput")
    wqkv = nc.dram_tensor("wqkv", [128, 6, NFT, DH], bf16,
                          kind="ExternalInput")
    bqkv = nc.dram_tensor("bqkv", [6, DH], f32, kind="ExternalInput")
    wout = nc.dram_tensor("wout", [128, 8, NFT, 256], bf16,
                          kind="ExternalInput")
    bout = nc.dram_tensor("bout", [D], f32, kind="ExternalInput")
    outT = nc.dram_tensor("outT", [D, ROWS], f32, kind="ExternalOutput")
    BCH = 512                # phase-B t-chunk width
    NBCH = T // BCH          # 4 chunks per (head, batch)
    NST = T // 128           # 16 s-tiles per batch

    with tile.TileContext(nc) as tc, ExitStack() as es:
        persist = es.enter_context(tc.tile_pool(name="persist", bufs=1))
        dram = es.enter_context(tc.tile_pool(name="dram", bufs=1,
                                             space="DRAM"))
        a2a_in = [dram.tile([NCORES, DH, ROWS], bf16, name=f"a2a_in{h}")
                  for h in range(HPC)]
        a2a_out = [dram.tile([NCORES, DH, ROWS], bf16, name=f"a2a_out{h}")
                   for h in range(HPC)]

        bqkv_sb = persist.tile([128, 6], f32)
        bv_sb = persist.tile([128, HPC * DH], f32)
        bout_sb = persist.tile([128, NFT], f32)
        ones_sb = persist.tile([128, 128], f32)
        ones_b = persist.tile([128, 128], bf16)
        ones_r = persist.tile([128, 128], f32r)

        nc.vector.memset(ones_sb, 1.0)
        nc.vector.tensor_copy(ones_b, ones_sb)
        nc.vector.tensor_copy(ones_r, ones_sb)

        xr0_sb = [persist.tile([128, ROWS], bf16, name=f"xr0_{c}")
                  for c in range(NCORES)]

        pcB = es.enter_context(tc.tile_pool(name="pcB", bufs=1))
        xr1_sb = [pcB.tile([128, ROWS], bf16, name=f"xr1_{c}")
                  for c in range(NCORES)]
        partial_sb = pcB.tile([128, NFT, ROWS], bf16)
        xr_sb = [xr0_sb, xr1_sb]

        mid = es.enter_context(tc.tile_pool(name="mid", bufs=1))
        qt_sb = [mid.tile([128, NT], bf16, name=f"qt{h}") for h in range(HPC)]
        kt_sb = [mid.tile([128, NT], bf16, name=f"kt{h}") for h in range(HPC)]
        v_sb = [mid.tile([128, B, NST, DH], bf16, name=f"v{h}")
                for h in range(HPC)]

        # ---------------- Phase A: QKV projection ----------------
        with tc.tile_pool(name="phaseA", bufs=1) as pa, \
             tc.tile_pool(name="xtp", bufs=2) as xtp, \
             tc.tile_pool(name="psA", bufs=2, space="PSUM") as psA:
            wqkv_sb = pa.tile([128, 6, NFT, DH], bf16)
            # JIT-ordered startup: interleave the first x chunk's pieces with
            # the weight slots in consumption order across both DMA queues
            nc.sync.dma_start(out=wqkv_sb[:, 0, 0:8, :],
                              in_=wqkv[:, 0, 0:8, :])

            first_q_act = None
            for tch in range(NACH):
                t0 = tch * ACH
                xt_tile = xtp.tile([128, NFT, ACH], bf16, tag="xt")
                if tch == 0:
                    for q in range(8):
                        eng = nc.scalar if q % 2 == 0 else nc.sync
                        eng.dma_start(
                            out=xt_tile[:, 2 * q:2 * q + 2, :],
                            in_=xT[:, 0, 2 * q:2 * q + 2, :])
                        if q == 1:
                            nc.sync.dma_start(out=wqkv_sb[:, 0, 8:16, :],
                                              in_=wqkv[:, 0, 8:16, :])
                        elif q == 3:
                            nc.sync.dma_start(out=wqkv_sb[:, 2, :, :],
                                              in_=wqkv[:, 2, :, :])
                            nc.sync.dma_start(
                                out=bqkv_sb, in_=bqkv[:, :].transpose([1, 0]))
                        elif q == 5:
                            nc.scalar.dma_start(out=wqkv_sb[:, 1, :, :],
                                                in_=wqkv[:, 1, :, :])
                        elif q == 7:
                            nc.scalar.dma_start(out=wqkv_sb[:, 3, :, :],
                                                in_=wqkv[:, 3, :, :])
                    nc.sync.dma_start(out=wqkv_sb[:, 4, :, :],
                                      in_=wqkv[:, 4, :, :])
                    nc.scalar.dma_start(out=wqkv_sb[:, 5, :, :],
                                        in_=wqkv[:, 5, :, :])
                    nc.sync.dma_start(out=bv_sb,
                                      in_=bqkv[4:6, :].flatten().unsqueeze(0)
                                      .to_broadcast([128, HPC * DH]))
                    nc.sync.dma_start(out=bout_sb,
                                      in_=bout.rearrange("(n p) -> p n",
                                                         p=128))
                else:
                    dma = nc.gpsimd.dma_start(out=xt_tile,
                                              in_=xT[:, tch, :, :])
                    if tch == 1 and first_q_act is not None:
                        # keep the chunk-1 prefetch off the DMA engines while
                        # chunk 0's critical pieces stream in
                        from concourse.bass import _add_dep_helper
                        _add_dep_helper(dma.ins, first_q_act.ins, sync=False,
                                        reason="xt1 after first Q chain")
                for h in range(HPC):
                    ps_q = psA.tile([128, ACH], f32, tag="psq")
                    for ft in range(NFT):
                        nc.tensor.matmul(
                            ps_q,
                            wqkv_sb[:, h, ft, :], xt_tile[:, ft, :],
                            start=(ft == 0), stop=(ft == NFT - 1))
                    act = nc.scalar.activation(
                        out=qt_sb[h][:, t0:t0 + ACH], in_=ps_q,
                        func=Act.Identity, bias=bqkv_sb[:, h:h + 1])
                    if tch == 0 and h == 0 and first_q_act is None:
                        first_q_act = act
                    ps_k = psA.tile([128, ACH], f32, tag="psk")
                    for ft in range(NFT):
                        nc.tensor.matmul(
                            ps_k,
                            wqkv_sb[:, 2 + h, ft, :], xt_tile[:, ft, :],
                            start=(ft == 0), stop=(ft == NFT - 1))
                    nc.scalar.activation(
                        out=kt_sb[h][:, t0:t0 + ACH], in_=ps_k,
                        func=Act.Identity, bias=bqkv_sb[:, 2 + h:3 + h])
                for st in range(ACH // 128):
                    ps_v = psA.tile([128, HPC * DH], f32, tag="psv")
                    for ft in range(NFT):
                        nc.tensor.matmul(
                            ps_v,
                            xt_tile[:, ft, st * 128:(st + 1) * 128],
                            wqkv_sb[:, 4:6, ft, :],
                            start=(ft == 0), stop=(ft == NFT - 1))
                    g = t0 + st * 128
                    b_idx, st_b = g // T, (g % T) // 128
                    for h in range(HPC):
                        nc.vector.tensor_add(
                            v_sb[h][:, b_idx, st_b, :],
                            ps_v[:, h * DH:(h + 1) * DH],
                            bv_sb[:, h * DH:(h + 1) * DH])

        wp_pool = es.enter_context(tc.tile_pool(name="wp", bufs=1))
        wpiece = [wp_pool.tile([128, NFT, 256], bf16, name=f"wp{p}")
                  for p in range(8)]
        for p in range(8):
            nc.gpsimd.dma_start(out=wpiece[p], in_=wout[:, p, :, :])

        def emit_c_chain(psc_pool, hh, fti, outp=None):
            ps_c = psc_pool.tile([128, ROWS], f32, tag="psc",
                                 name=f"psc{hh}_{fti}")
            for c in range(NCORES):
                nc.tensor.matmul(
                    ps_c,
                    wpiece[fti // 2][:, HPC * c + hh,
                                     (fti % 2) * 128:
                                     (fti % 2) * 128 + 128],
                    xr_sb[hh][c],
                    start=(c == 0), stop=(c == NCORES - 1))
            if hh == 0:
                nc.scalar.activation(out=partial_sb[:, fti, :],
                                     in_=ps_c, func=Act.Identity,
                                     bias=bout_sb[:, fti:fti + 1])
            else:
                out_sb = outp.tile([128, ROWS], f32, tag="ob")
                nc.vector.tensor_add(out_sb, ps_c,
                                     partial_sb[:, fti, :])
                nc.sync.dma_start(
                    out=outT[fti * 128:(fti + 1) * 128, :],
                    in_=out_sb)

        # ---------------- Phase B: attention (h outer, split A2A) --------
        # Software pipeline: chunk k's AV matmuls interleave with chunk k+1's
        # S matmuls so PE stays busy while ACT works through the exps.
        with tc.tile_pool(name="ptp", bufs=2) as ptp, \
             tc.tile_pool(name="wkB", bufs=3) as wkB, \
             tc.tile_pool(name="psS", bufs=2, space="PSUM") as psS, \
             tc.tile_pool(name="psO", bufs=3, space="PSUM") as psO, \
             tc.tile_pool(name="psX", bufs=1, space="PSUM") as psX:

            def emit_av(pend, st_list):
                h, b, j, pt_p = pend["h"], pend["b"], pend["j"], pend["pt"]
                if pend["ps_o"] is None:
                    ps_o = psO.tile([128, BCH], f32, tag="pso",
                                    name=f"pso{h}_{j}")
                    pend["ps_o"] = ps_o
                for st in st_list:
                    nc.tensor.matmul(
                        pend["ps_o"],
                        v_sb[h][:, b, st, :], pt_p[:, st, :],
                        start=(st == 0), stop=(st == NST - 1))

            def emit_tree_step(pend, step):
                h, j, pt_p = pend["h"], pend["j"], pend["pt"]
                if step == 0:
                    acc4 = wkB.tile([128, 4, BCH], bf16, tag="acc4", bufs=2,
                                    name=f"acc4_{h}_{j}")
                    pend["acc4"] = acc4
                acc4 = pend["acc4"]
                if step == 0:
                    nc.vector.tensor_add(acc4, pt_p[:, 0:4, :],
                                         pt_p[:, 4:8, :])
                elif step == 1:
                    nc.vector.tensor_add(acc4, acc4, pt_p[:, 8:12, :])
                elif step == 2:
                    nc.vector.tensor_add(acc4, acc4, pt_p[:, 12:16, :])
                elif step == 3:
                    nc.vector.tensor_add(acc4[:, 0:2, :], acc4[:, 0:2, :],
                                         acc4[:, 2:4, :])
                else:
                    nc.vector.tensor_add(acc4[:, 0, :], acc4[:, 0, :],
                                         acc4[:, 1, :])

            def emit_epilogue(pend):
                h, b, j, pt_p = pend["h"], pend["b"], pend["j"], pend["pt"]
                for step in range(pend["tree_step"], 5):
                    emit_tree_step(pend, step)
                acc4 = pend["acc4"]
                ps_den = psX.tile([128, BCH], f32, tag="psx",
                                  name=f"psden{h}_{j}")
                nc.tensor.matmul(ps_den[0:1, :], ones_b[:, 0:1],
                                 acc4[:, 0, :],
                                 start=True, stop=True)
                den_sb = wkB.tile([1, BCH], f32, tag="densb",
                                  name=f"den{h}_{j}")
                nc.vector.reciprocal_approx_fast(den_sb[0:1, :],
                                                 ps_den[0:1, :])
                den_r = wkB.tile([1, BCH], f32r, tag="denr",
                                 name=f"denr{h}_{j}")
                nc.vector.tensor_copy(den_r[0:1, :], den_sb[0:1, :])
                ps_rb = psX.tile([128, BCH], f32, tag="psx",
                                 name=f"psrb{h}_{j}")
                nc.tensor.matmul(ps_rb, ones_r[0:1, :], den_r[0:1, :],
                                 start=True, stop=True)
                rb_sb = wkB.tile([128, BCH], f32, tag="rb",
                                 name=f"rb{h}_{j}")
                nc.vector.tensor_copy(rb_sb, ps_rb)
                obuf = wkB.tile([128, BCH], bf16, tag="obuf",
                                name=f"obuf{h}_{j}")
                nc.vector.tensor_mul(obuf, pend["ps_o"], rb_sb)
                nc.sync.dma_start(out=a2a_in[h][j, :, :], in_=obuf)

            def emit_a2a(h):
                nc.gpsimd.collective_compute(
                    "AllToAll", mybir.AluOpType.bypass,
                    replica_groups=[list(range(NCORES))],
                    ins=[a2a_in[h].opt()], outs=[a2a_out[h].opt()])
                if h == 0:
                    # xr0 staging runs under head 1's attention; the gpsimd
                    # queue is idle in phase B so its wait on A2A#1 blocks
                    # nothing. Per-core pieces keep each DMA contiguous.
                    for c in range(NCORES):
                        nc.gpsimd.dma_start(out=xr0_sb[c],
                                            in_=a2a_out[0][c, :, :])

            pending = None
            for h in range(HPC):
                for b in range(B):
                    for tc2 in range(NBCH):
                        t0 = b * T + tc2 * BCH
                        pt = ptp.tile([128, NST, BCH], bf16, tag="pt",
                                      name=f"pt{h}_{b}_{tc2}")
                        cur = {"h": h, "b": b, "j": b * NBCH + tc2,
                               "pt": pt, "ps_o": None, "tree_step": 0}
                        for sg in range(NST // 2):
                            ps_s = psS.tile([128, 2, BCH], f32, tag="pss",
                                            name=f"pss{h}_{b}_{tc2}_{sg}")
                            for si in range(2):
                                st = sg * 2 + si
                                nc.tensor.matmul(
                                    ps_s[:, si, :],
                                    kt_sb[h][:,
                                             b * T + st * 128:
                                             b * T + (st + 1) * 128],
                                    qt_sb[h][:, t0:t0 + BCH],
                                    start=True, stop=True)
                            nc.scalar.activation(
                                out=pt[:, sg * 2:(sg + 1) * 2, :],
                                in_=ps_s, func=Act.Exp, scale=SCALE)
                            if pending is not None:
                                emit_av(pending, [sg * 2, sg * 2 + 1])
                                if sg >= 3 and pending["tree_step"] < 5:
                                    emit_tree_step(pending,
                                                   pending["tree_step"])
                                    pending["tree_step"] += 1
                            last = (h == HPC - 1 and b == B - 1
                                    and tc2 == NBCH - 1)
                            if last and sg > 0:
                                # drain chunk: AV rides one batch behind exp
                                emit_av(cur, [sg * 2 - 2, sg * 2 - 1])
                        if pending is not None:
                            emit_epilogue(pending)
                            if pending["h"] == 0 and \
                                    pending["j"] == B * NBCH - 1:
                                # head 0 fully drained: its AllToAll issues
                                # here, one chunk into head 1's attention, so
                                # h1's S matmuls are never queued behind the
                                # epilogue matvecs
                                emit_a2a(0)
                        pending = cur
            # final drain and head-1 collective
            emit_av(pending, [NST - 2, NST - 1])
            emit_epilogue(pending)
            emit_a2a(1)
        # ---------------- Phase C: output projection ----------------
        outp = es.enter_context(tc.tile_pool(name="outp", bufs=3))
        psC = es.enter_context(tc.tile_pool(name="psC", bufs=8, space="PSUM"))
        for c in range(NCORES):
            nc.gpsimd.dma_start(out=xr1_sb[c], in_=a2a_out[1][c, :, :])
        for hh in range(HPC):
            for fti in range(NFT):
                emit_c_chain(psC, hh, fti, outp)

    nc.compile()
    return nc


def get_nc():
    if "nc" not in _CACHE:
        _CACHE["nc"] = _build()
    return _CACHE["nc"]


def make_in_maps(query, W_qkv, b_qkv, W_out, b_out):
    import ml_dtypes
    bf = ml_dtypes.bfloat16
    query = np.asarray(query, dtype=np.float32)
    W_qkv = np.asarray(W_qkv, dtype=np.float32)
    b_qkv = np.asarray(b_qkv, dtype=np.float32)
    W_out_bf = (np.asarray(W_out, dtype=np.float32)
                .astype(bf)
                .reshape(NFT, 128, 8, 256).transpose(1, 2, 0, 3))
    W_out_bf = np.ascontiguousarray(W_out_bf)  # [128, 8, NFT, 256]
    b_out = np.ascontiguousarray(np.asarray(b_out, dtype=np.float32))

    ACH = 512
    x = query.reshape(NT, D).astype(bf)
    xT = x.T.reshape(NFT, 128, NT // ACH, ACH).transpose(1, 2, 0, 3)
    xT = np.ascontiguousarray(xT)          # [128, NACH, NFT, ACH]

    in_maps = []
    for c in range(NCORES):
        h0, h1 = HPC * c, HPC * c + 1
        cols, bcols = [], []
        for blk in range(3):  # q, k, v blocks of W_qkv
            for h in (h0, h1):
                sl = slice(blk * D + h * DH, blk * D + (h + 1) * DH)
                cols.append(W_qkv[:, sl])
                bcols.append(b_qkv[sl])
        wq = (np.stack(cols, axis=0)            # [6, D, DH]
              .reshape(6, NFT, 128, DH).transpose(2, 0, 1, 3))
        in_maps.append({
            "xT": xT,
            "wqkv": np.ascontiguousarray(wq.astype(bf)),  # [128, 6, NFT, DH]
            "bqkv": np.ascontiguousarray(np.stack(bcols, axis=0)),
            "wout": W_out_bf,
            "bout": b_out,
        })
    return in_maps


def kernel(query, key, value, W_qkv, b_qkv, W_out, b_out):
    from concourse.bass_utils import run_bass_kernel_spmd

    nc = get_nc()
    in_maps = make_in_maps(query, W_qkv, b_qkv, W_out, b_out)
    res = run_bass_kernel_spmd(nc, in_maps, list(range(NCORES)))
    out = np.empty((NT, D), dtype=np.float32)
    for c in range(NCORES):
        out[c * ROWS:(c + 1) * ROWS, :] = res.results[c]["outT"].T
    return out.reshape(B, T, D)


# revision 24
# speedup vs baseline: 1.0434x; 1.0086x over previous
"""Multi-head attention (B=2, T=2048, D=2048, H=16) on 8 trn2 NeuronCores.

Sharding: tensor-parallel over heads. Core c owns heads {2c, 2c+1}:
  - QKV projection for its 2 heads (Q^T/K^T in [dh, t] layout, V in [t, dh]).
  - Attention per (head, batch), computed as S^T = K^T.T Q^T so softmax probs
    land in [s, t] layout and feed the AV matmul directly (no transposes).
    Softmax skips max-subtraction (scores are O(15) here; exp stays well
    inside fp32 range). The denominator is tree-accumulated on DVE in bf16,
    partition-reduced by a ones-matvec on PE, reciprocated at [1, t] and
    broadcast back over partitions with a rank-1 PE matmul.
  - Two per-head AllToAlls redistribute attention outputs from head-sharded
    [dh, t] blocks to row-sharded x^T [D, 512] per core; the first overlaps
    with the second head's attention, the second hides under the hh=0 half
    of the output projection (which only needs AllToAll #1's data). The xr
    staging is split into 8 contiguous per-core DMAs on the gpsimd queue so
    no strided descriptor storm sits on the critical path.
  - Each core then computes 512 rows of the output projection with the full
    W_out, preloaded into SBUF during phase B when DMA is otherwise idle.
Host assembles the 8 row-shards (each returned transposed) into [B, T, D].

All matmul operands are bf16: same PE rate as f32r but 1.5x faster
LDWEIGHTS, half the DMA/SBUF footprint, and 2x DVE throughput for the
softmax-denominator tree; accumulation stays fp32 in PSUM. Startup DMAs are
JIT-ordered across the sync/scalar queues in first-chain consumption order
(the first ~30us are DMA-bandwidth-bound).
"""

import numpy as np

D = 2048
H = 16
DH = 128
B = 2
T = 2048
NT = B * T            # 4096 flattened rows
NCORES = 8
HPC = H // NCORES     # heads per core = 2
ROWS = NT // NCORES   # output rows per core = 512
NFT = D // 128        # 16 feature tiles
SCALE = float(np.sqrt(np.float32(DH)) / np.sqrt(np.float32(D)))  # 0.25

_CACHE = {}


def _build():
    from contextlib import ExitStack

    import concourse.bass as bass  # noqa: F401
    import concourse.mybir as mybir
    import concourse.tile as tile
    from concourse import bacc

    f32 = mybir.dt.float32
    f32r = mybir.dt.float32r
    bf16 = mybir.dt.bfloat16
    Act = mybir.ActivationFunctionType

    nc = bacc.Bacc("TRN2", target_bir_lowering=False, debug=False,
                   num_devices=NCORES)

    ACH = 512                # phase-A t-chunk width
    NACH = NT // ACH         # 8 chunks

    # inputs are pre-arranged on the host into SBUF-image layouts so every
    # load is a fully contiguous per-partition DMA
    xT = nc.dram_tensor("xT", [128, NACH, NFT, ACH], bf16,
                        kind="ExternalInput")
    wqkv = nc.dram_tensor("wqkv", [128, 6, NFT, DH], bf16,
                          kind="ExternalInput")
    bqkv = nc.dram_tensor("bqkv", [6, DH], f32, kind="ExternalInput")
    wout = nc.dram_tensor("wout", [128, 8, NFT, 256], bf16,
                          kind="ExternalInput")
    bout = nc.dram_tensor("bout", [D], f32, kind="ExternalInput")
    outT = nc.dram_tensor("outT", [D, ROWS], f32, kind="ExternalOutput")
    BCH = 512                # phase-B t-chunk width
    NBCH = T // BCH          # 4 chunks per (head, batch)
    NST = T // 128           # 16 s-tiles per batch

    with tile.TileContext(nc) as tc, ExitStack() as es:
        persist = es.enter_context(tc.tile_pool(name="persist", bufs=1))
        dram = es.enter_context(tc.tile_pool(name="dram", bufs=1,
                                             space="DRAM"))
        a2a_in = [dram.tile([NCORES, DH, ROWS], bf16, name=f"a2a_in{h}")
                  for h in range(HPC)]
        a2a_out = [dram.tile([NCORES, DH, ROWS], bf16, name=f"a2a_out{h}")
                   for h in range(HPC)]

        bqkv_sb = persist.tile([128, 6], f32)
        bv_sb = persist.tile([128, HPC * DH], f32)
        bout_sb = persist.tile([128, NFT], f32)
        ones_sb = persist.tile([128, 128], f32)
        ones_b = persist.tile([128, 128], bf16)
        ones_r = persist.tile([128, 128], f32r)

        nc.vector.memset(ones_sb, 1.0)
        nc.vector.tensor_copy(ones_b, ones_sb)
        nc.vector.tensor_copy(ones_r, ones_sb)

        xr0_sb = [persist.tile([128, ROWS], bf16, name=f"xr0_{c}")
                  for c in range(NCORES)]

        pcB = es.enter_context(tc.tile_pool(name="pcB", bufs=1))
        xr1_sb = [pcB.tile([128, ROWS], bf16, name=f"xr1_{c}")
                  for c in range(NCORES)]
        partial_sb = pcB.tile([128, NFT, ROWS], bf16)
        xr_sb = [xr0_sb, xr1_sb]

        mid = es.enter_context(tc.tile_pool(name="mid", bufs=1))
        qt_sb = [mid.tile([128, NT], bf16, name=f"qt{h}") for h in range(HPC)]
        kt_sb = [mid.tile([128, NT], bf16, name=f"kt{h}") for h in range(HPC)]
        v_sb = [mid.tile([128, B, NST, DH], bf16, name=f"v{h}")
                for h in range(HPC)]

        # ---------------- Phase A: QKV projection ----------------
        with tc.tile_pool(name="phaseA", bufs=1) as pa, \
             tc.tile_pool(name="xtp", bufs=2) as xtp, \
             tc.tile_pool(name="psA", bufs=2, space="PSUM") as psA:
            wqkv_sb = pa.tile([128, 6, NFT, DH], bf16)
            # JIT-ordered startup: interleave the first x chunk's pieces with
            # the weight slots in consumption order across both DMA queues
            nc.sync.dma_start(out=wqkv_sb[:, 0, 0:8, :],
                              in_=wqkv[:, 0, 0:8, :])

            first_q_act = None
            for tch in range(NACH):
                t0 = tch * ACH
                xt_tile = xtp.tile([128, NFT, ACH], bf16, tag="xt")
                if tch == 0:
                    for q in range(8):
                        eng = nc.scalar if q % 2 == 0 else nc.sync
                        eng.dma_start(
                            out=xt_tile[:, 2 * q:2 * q + 2, :],
                            in_=xT[:, 0, 2 * q:2 * q + 2, :])
                        if q == 1:
                            nc.sync.dma_start(out=wqkv_sb[:, 0, 8:16, :],
                                              in_=wqkv[:, 0, 8:16, :])
                        elif q == 3:
                            nc.sync.dma_start(out=wqkv_sb[:, 2, :, :],
                                              in_=wqkv[:, 2, :, :])
                            nc.sync.dma_start(
                                out=bqkv_sb, in_=bqkv[:, :].transpose([1, 0]))
                        elif q == 5:
                            nc.scalar.dma_start(out=wqkv_sb[:, 1, :, :],
                                                in_=wqkv[:, 1, :, :])
                        elif q == 7:
                            nc.scalar.dma_start(out=wqkv_sb[:, 3, :, :],
                                                in_=wqkv[:, 3, :, :])
                    nc.sync.dma_start(out=wqkv_sb[:, 4, :, :],
                                      in_=wqkv[:, 4, :, :])
                    nc.scalar.dma_start(out=wqkv_sb[:, 5, :, :],
                                        in_=wqkv[:, 5, :, :])
                    nc.sync.dma_start(out=bv_sb,
                                      in_=bqkv[4:6, :].flatten().unsqueeze(0)
                                      .to_broadcast([128, HPC * DH]))
                    nc.sync.dma_start(out=bout_sb,
                                      in_=bout.rearrange("(n p) -> p n",
                                                         p=128))
                else:
                    dma = nc.gpsimd.dma_start(out=xt_tile,
                                              in_=xT[:, tch, :, :])
                    if tch == 1 and first_q_act is not None:
                        # keep the chunk-1 prefetch off the DMA engines while
                        # chunk 0's critical pieces stream in
                        from concourse.bass import _add_dep_helper
                        _add_dep_helper(dma.ins, first_q_act.ins, sync=False,
                                        reason="xt1 after first Q chain")
                if tch == 0:
                    # chunk 0 is DMA-starved: interleave the four Q/K chains
                    # at half-chain granularity so the first-arrived x pieces
                    # feed ~7us of PE work while the rest stream in
                    ps_qk = [psA.tile([128, ACH], f32,
                                      tag=("psq" if i < 2 else "psk"),
                                      name=f"psqk0_{i}")
                             for i in range(4)]
                    for half in range(2):
                        for i, slot in enumerate((0, 1, 2, 3)):
                            for ft in range(half * 8, half * 8 + 8):
                                nc.tensor.matmul(
                                    ps_qk[i],
                                    wqkv_sb[:, slot, ft, :],
                                    xt_tile[:, ft, :],
                                    start=(ft == 0), stop=(ft == NFT - 1))
                    for h in range(HPC):
                        act = nc.scalar.activation(
                            out=qt_sb[h][:, t0:t0 + ACH], in_=ps_qk[h],
                            func=Act.Identity, bias=bqkv_sb[:, h:h + 1])
                        if first_q_act is None:
                            first_q_act = act
                        nc.scalar.activation(
                            out=kt_sb[h][:, t0:t0 + ACH], in_=ps_qk[2 + h],
                            func=Act.Identity, bias=bqkv_sb[:, 2 + h:3 + h])
                else:
                    for h in range(HPC):
                        ps_q = psA.tile([128, ACH], f32, tag="psq")
                        for ft in range(NFT):
                            nc.tensor.matmul(
                                ps_q,
                                wqkv_sb[:, h, ft, :], xt_tile[:, ft, :],
                                start=(ft == 0), stop=(ft == NFT - 1))
                        nc.scalar.activation(
                            out=qt_sb[h][:, t0:t0 + ACH], in_=ps_q,
                            func=Act.Identity, bias=bqkv_sb[:, h:h + 1])
                        ps_k = psA.tile([128, ACH], f32, tag="psk")
                        for ft in range(NFT):
                            nc.tensor.matmul(
                                ps_k,
                                wqkv_sb[:, 2 + h, ft, :], xt_tile[:, ft, :],
                                start=(ft == 0), stop=(ft == NFT - 1))
                        nc.scalar.activation(
                            out=kt_sb[h][:, t0:t0 + ACH], in_=ps_k,
                            func=Act.Identity, bias=bqkv_sb[:, 2 + h:3 + h])
                for st in range(ACH // 128):
                    ps_v = psA.tile([128, HPC * DH], f32, tag="psv")
                    for ft in range(NFT):
                        nc.tensor.matmul(
                            ps_v,
                            xt_tile[:, ft, st * 128:(st + 1) * 128],
                            wqkv_sb[:, 4:6, ft, :],
                            start=(ft == 0), stop=(ft == NFT - 1))
                    g = t0 + st * 128
                    b_idx, st_b = g // T, (g % T) // 128
                    for h in range(HPC):
                        nc.vector.tensor_add(
                            v_sb[h][:, b_idx, st_b, :],
                            ps_v[:, h * DH:(h + 1) * DH],
                            bv_sb[:, h * DH:(h + 1) * DH])

        wp_pool = es.enter_context(tc.tile_pool(name="wp", bufs=1))
        wpiece = [wp_pool.tile([128, NFT, 256], bf16, name=f"wp{p}")
                  for p in range(8)]
        for p in range(8):
            nc.gpsimd.dma_start(out=wpiece[p], in_=wout[:, p, :, :])

        def emit_c_chain(psc_pool, hh, fti, outp=None):
            ps_c = psc_pool.tile([128, ROWS], f32, tag="psc",
                                 name=f"psc{hh}_{fti}")
            for c in range(NCORES):
                nc.tensor.matmul(
                    ps_c,
                    wpiece[fti // 2][:, HPC * c + hh,
                                     (fti % 2) * 128:
                                     (fti % 2) * 128 + 128],
                    xr_sb[hh][c],
                    start=(c == 0), stop=(c == NCORES - 1))
            if hh == 0:
                nc.scalar.activation(out=partial_sb[:, fti, :],
                                     in_=ps_c, func=Act.Identity,
                                     bias=bout_sb[:, fti:fti + 1])
            else:
                out_sb = outp.tile([128, ROWS], f32, tag="ob")
                nc.vector.tensor_add(out_sb, ps_c,
                                     partial_sb[:, fti, :])
                nc.sync.dma_start(
                    out=outT[fti * 128:(fti + 1) * 128, :],
                    in_=out_sb)

        # ---------------- Phase B: attention (h outer, split A2A) --------
        # Software pipeline: chunk k's AV matmuls interleave with chunk k+1's
        # S matmuls so PE stays busy while ACT works through the exps.
        with tc.tile_pool(name="ptp", bufs=2) as ptp, \
             tc.tile_pool(name="wkB", bufs=3) as wkB, \
             tc.tile_pool(name="psS", bufs=2, space="PSUM") as psS, \
             tc.tile_pool(name="psO", bufs=3, space="PSUM") as psO, \
             tc.tile_pool(name="psX", bufs=1, space="PSUM") as psX:

            def emit_av(pend, st_list):
                h, b, j, pt_p = pend["h"], pend["b"], pend["j"], pend["pt"]
                if pend["ps_o"] is None:
                    ps_o = psO.tile([128, BCH], f32, tag="pso",
                                    name=f"pso{h}_{j}")
                    pend["ps_o"] = ps_o
                for st in st_list:
                    nc.tensor.matmul(
                        pend["ps_o"],
                        v_sb[h][:, b, st, :], pt_p[:, st, :],
                        start=(st == 0), stop=(st == NST - 1))

            def emit_tree_step(pend, step):
                h, j, pt_p = pend["h"], pend["j"], pend["pt"]
                if step == 0:
                    acc4 = wkB.tile([128, 4, BCH], bf16, tag="acc4", bufs=2,
                                    name=f"acc4_{h}_{j}")
                    pend["acc4"] = acc4
                acc4 = pend["acc4"]
                if step == 0:
                    nc.vector.tensor_add(acc4, pt_p[:, 0:4, :],
                                         pt_p[:, 4:8, :])
                elif step == 1:
                    nc.vector.tensor_add(acc4, acc4, pt_p[:, 8:12, :])
                elif step == 2:
                    nc.vector.tensor_add(acc4, acc4, pt_p[:, 12:16, :])
                elif step == 3:
                    nc.vector.tensor_add(acc4[:, 0:2, :], acc4[:, 0:2, :],
                                         acc4[:, 2:4, :])
                else:
                    nc.vector.tensor_add(acc4[:, 0, :], acc4[:, 0, :],
                                         acc4[:, 1, :])

            def emit_epilogue(pend):
                h, b, j, pt_p = pend["h"], pend["b"], pend["j"], pend["pt"]
                for step in range(pend["tree_step"], 5):
                    emit_tree_step(pend, step)
                acc4 = pend["acc4"]
                ps_den = psX.tile([128, BCH], f32, tag="psx",
                                  name=f"psden{h}_{j}")
                nc.tensor.matmul(ps_den[0:1, :], ones_b[:, 0:1],
                                 acc4[:, 0, :],
                                 start=True, stop=True)
                den_sb = wkB.tile([1, BCH], f32, tag="densb",
                                  name=f"den{h}_{j}")
                nc.vector.reciprocal_approx_fast(den_sb[0:1, :],
                                                 ps_den[0:1, :])
                den_r = wkB.tile([1, BCH], f32r, tag="denr",
                                 name=f"denr{h}_{j}")
                nc.vector.tensor_copy(den_r[0:1, :], den_sb[0:1, :])
                ps_rb = psX.tile([128, BCH], f32, tag="psx",
                                 name=f"psrb{h}_{j}")
                nc.tensor.matmul(ps_rb, ones_r[0:1, :], den_r[0:1, :],
                                 start=True, stop=True)
                rb_sb = wkB.tile([128, BCH], f32, tag="rb",
                                 name=f"rb{h}_{j}")
                nc.vector.tensor_copy(rb_sb, ps_rb)
                obuf = wkB.tile([128, BCH], bf16, tag="obuf",
                                name=f"obuf{h}_{j}")
                nc.vector.tensor_mul(obuf, pend["ps_o"], rb_sb)
                nc.sync.dma_start(out=a2a_in[h][j, :, :], in_=obuf)

            def emit_a2a(h):
                nc.gpsimd.collective_compute(
                    "AllToAll", mybir.AluOpType.bypass,
                    replica_groups=[list(range(NCORES))],
                    ins=[a2a_in[h].opt()], outs=[a2a_out[h].opt()])
                if h == 0:
                    # xr0 staging runs under head 1's attention; the gpsimd
                    # queue is idle in phase B so its wait on A2A#1 blocks
                    # nothing. Per-core pieces keep each DMA contiguous.
                    for c in range(NCORES):
                        nc.gpsimd.dma_start(out=xr0_sb[c],
                                            in_=a2a_out[0][c, :, :])

            pending = None
            for h in range(HPC):
                for b in range(B):
                    for tc2 in range(NBCH):
                        t0 = b * T + tc2 * BCH
                        pt = ptp.tile([128, NST, BCH], bf16, tag="pt",
                                      name=f"pt{h}_{b}_{tc2}")
                        cur = {"h": h, "b": b, "j": b * NBCH + tc2,
                               "pt": pt, "ps_o": None, "tree_step": 0}
                        for sg in range(NST // 2):
                            ps_s = psS.tile([128, 2, BCH], f32, tag="pss",
                                            name=f"pss{h}_{b}_{tc2}_{sg}")
                            for si in range(2):
                                st = sg * 2 + si
                                nc.tensor.matmul(
                                    ps_s[:, si, :],
                                    kt_sb[h][:,
                                             b * T + st * 128:
                                             b * T + (st + 1) * 128],
                                    qt_sb[h][:, t0:t0 + BCH],
                                    start=True, stop=True)
                            nc.scalar.activation(
                                out=pt[:, sg * 2:(sg + 1) * 2, :],
                                in_=ps_s, func=Act.Exp, scale=SCALE)
                            if pending is not None:
                                emit_av(pending, [sg * 2, sg * 2 + 1])
                                if sg >= 3 and pending["tree_step"] < 5:
                                    emit_tree_step(pending,
                                                   pending["tree_step"])
                                    pending["tree_step"] += 1
                            last = (h == HPC - 1 and b == B - 1
                                    and tc2 == NBCH - 1)
                            if last and sg > 0:
                                # drain chunk: AV rides one batch behind exp
                                emit_av(cur, [sg * 2 - 2, sg * 2 - 1])
                        if pending is not None:
                            emit_epilogue(pending)
                            if pending["h"] == 0 and \
                                    pending["j"] == B * NBCH - 1:
                                # head 0 fully drained: its AllToAll issues
                                # here, one chunk into head 1's attention, so
                                # h1's S matmuls are never queued behind the
                                # epilogue matvecs
                                emit_a2a(0)
                        pending = cur
            # final drain and head-1 collective
            emit_av(pending, [NST - 2, NST - 1])
            emit_epilogue(pending)
            emit_a2a(1)
        # ---------------- Phase C: output projection ----------------
        outp = es.enter_context(tc.tile_pool(name="outp", bufs=3))
        psC = es.enter_context(tc.tile_pool(name="psC", bufs=8, space="PSUM"))
        for c in range(NCORES):
            nc.gpsimd.dma_start(out=xr1_sb[c], in_=a2a_out[1][c, :, :])
        for hh in range(HPC):
            for fti in range(NFT):
                emit_c_chain(psC, hh, fti, outp)

    nc.compile()
    return nc


def get_nc():
    if "nc" not in _CACHE:
        _CACHE["nc"] = _build()
    return _CACHE["nc"]


def make_in_maps(query, W_qkv, b_qkv, W_out, b_out):
    import ml_dtypes
    bf = ml_dtypes.bfloat16
    query = np.asarray(query, dtype=np.float32)
    W_qkv = np.asarray(W_qkv, dtype=np.float32)
    b_qkv = np.asarray(b_qkv, dtype=np.float32)
    W_out_bf = (np.asarray(W_out, dtype=np.float32)
                .astype(bf)
                .reshape(NFT, 128, 8, 256).transpose(1, 2, 0, 3))
    W_out_bf = np.ascontiguousarray(W_out_bf)  # [128, 8, NFT, 256]
    b_out = np.ascontiguousarray(np.asarray(b_out, dtype=np.float32))

    ACH = 512
    x = query.reshape(NT, D).astype(bf)
    xT = x.T.reshape(NFT, 128, NT // ACH, ACH).transpose(1, 2, 0, 3)
    xT = np.ascontiguousarray(xT)          # [128, NACH, NFT, ACH]

    in_maps = []
    for c in range(NCORES):
        h0, h1 = HPC * c, HPC * c + 1
        cols, bcols = [], []
        for blk in range(3):  # q, k, v blocks of W_qkv
            for h in (h0, h1):
                sl = slice(blk * D + h * DH, blk * D + (h + 1) * DH)
                cols.append(W_qkv[:, sl])
                bcols.append(b_qkv[sl])
        wq = (np.stack(cols, axis=0)            # [6, D, DH]
              .reshape(6, NFT, 128, DH).transpose(2, 0, 1, 3))
        in_maps.append({
            "xT": xT,
            "wqkv": np.ascontiguousarray(wq.astype(bf)),  # [128, 6, NFT, DH]
            "bqkv": np.ascontiguousarray(np.stack(bcols, axis=0)),
            "wout": W_out_bf,
            "bout": b_out,
        })
    return in_maps


def kernel(query, key, value, W_qkv, b_qkv, W_out, b_out):
    from concourse.bass_utils import run_bass_kernel_spmd

    nc = get_nc()
    in_maps = make_in_maps(query, W_qkv, b_qkv, W_out, b_out)
    res = run_bass_kernel_spmd(nc, in_maps, list(range(NCORES)))
    out = np.empty((NT, D), dtype=np.float32)
    for c in range(NCORES):
        out[c * ROWS:(c + 1) * ROWS, :] = res.results[c]["outT"].T
    return out.reshape(B, T, D)
